# revision 20
# baseline (speedup 1.0000x reference)
"""Trainium2 Bass kernel for nn_Decoder: pointer-network sequential decode with
Gumbel-max categorical sampling, data-parallel over batch across 8 NeuronCores.

Contract: kernel(**inputs) takes FULL unsharded inputs (as produced by
setup_inputs()) and returns (cell_log_prob [B], cell_reward [B], cell_action [B,S])
matching the reference bit-for-bit on sampled actions.

Strategy:
 - The jax.random.categorical noise is data-independent: precompute the exact
   Gumbel noise host-side (jax-CPU subprocess for bit-exactness; numpy threefry
   fallback) and feed it to the device.
 - Per core (64 batch rows): the sequential 64-step decode runs on-device:
   A = tgt + q (DVE), T = tanh(A) (ACT), u = wv.T @ T (PE), redistribute u via
   DMA, sample via max/max_index, gather next-q rows via indirect DMA from a
   precomputed G = cc @ (Wv_bot @ W1) table, q = cst + gathered^T (PE transpose).
 - log-prob and reward are pure outputs (not in the recurrence); the device
   streams out tanh(u) per step and the host finishes log_softmax/reward in
   fp32 numpy.
"""

import os
import subprocess
import sys
import tempfile

import numpy as np

B, S, E, H = 512, 64, 128, 128
NC = 8
BL = B // NC  # 64 rows per core
C_CLIP = np.float32(10.0)
MASK_NEG = np.float32(1e8)

_F32R = os.environ.get("KERNEL_F32R", "0") == "1"  # full-rate fp32 matmuls
_NSPLIT = int(os.environ.get("KERNEL_NSPLIT", "1"))   # add/tanh split count
_ACT_COPIES = int(os.environ.get("KERNEL_ACT_COPIES", "4"))  # of 8 u-chunk copies
_MERGE = os.environ.get("KERNEL_MERGE", "1") == "1"


# ----------------------------------------------------------------------------
# Gumbel noise replication (bit-exact with jax.random.categorical's noise)
# ----------------------------------------------------------------------------

_GUMBEL_SUBPROC = r"""
import numpy as np, sys
import jax, jax.numpy as jnp
key = jax.random.key(1)
keys = jax.random.split(key, %d)
g = np.stack([np.asarray(jax.random.gumbel(keys[t], (%d, %d), jnp.float32))
              for t in range(%d)])
np.save(sys.argv[1], g)
""" % (S, B, S, S)


def _rotl(x, r):
    return ((x << np.uint32(r)) | (x >> np.uint32(32 - r))).astype(np.uint32)


def _threefry2x32(k0, k1, x0, x1):
    ks0, ks1 = np.uint32(k0), np.uint32(k1)
    ks2 = np.uint32(np.uint32(0x1BD11BDA) ^ ks0 ^ ks1)
    x0 = (x0 + ks0).astype(np.uint32)
    x1 = (x1 + ks1).astype(np.uint32)
    sched = [(ks1, ks2, 1), (ks2, ks0, 2), (ks0, ks1, 3), (ks1, ks2, 4), (ks2, ks0, 5)]
    rotsets = [(13, 15, 26, 6), (17, 29, 16, 24)]
    for i, (ka, kb, c) in enumerate(sched):
        for r in rotsets[i % 2]:
            x0 = (x0 + x1).astype(np.uint32)
            x1 = (_rotl(x1, r) ^ x0).astype(np.uint32)
        x0 = (x0 + ka).astype(np.uint32)
        x1 = (x1 + kb + np.uint32(c)).astype(np.uint32)
    return x0, x1


def _threefry_split(kd, n):
    # jax.random.split: bits = threefry_random_bits(key, 32, (n*2,)) -> reshape [n,2]
    bits = _random_bits32(kd, n * 2)
    return bits.reshape(n, 2)


def _random_bits32(kd, n):
    x0 = np.zeros(n, np.uint32)
    x1 = np.arange(n, dtype=np.uint32)
    r0, r1 = _threefry2x32(kd[0], kd[1], x0, x1)
    return (r0 ^ r1).astype(np.uint32)


def _gumbel_numpy():
    # key(1) -> key_data is [0, 1] (threefry_seed packs the int into 2 words)
    base = np.array([0, 1], dtype=np.uint32)
    keys = _threefry_split(base, S)
    out = np.empty((S, B, S), np.float32)
    tiny = np.float32(np.finfo(np.float32).tiny)
    for t in range(S):
        bits = _random_bits32(keys[t], B * S)
        fb = ((bits >> np.uint32(9)) | np.uint32(0x3F800000)).astype(np.uint32)
        f = fb.view(np.float32) - np.float32(1.0)
        u = np.maximum(tiny, (f * (np.float32(1.0) - tiny) + tiny).astype(np.float32))
        out[t] = (-np.log(-np.log(u, dtype=np.float32), dtype=np.float32)).reshape(B, S)
    return out


_NOISE_CACHE = []


def _gumbel_noise():
    """[S, B, S] float32 — exactly jax.random.gumbel(keys[t], (B, S)) on XLA-CPU."""
    if _NOISE_CACHE:
        return _NOISE_CACHE[0]
    g = _gumbel_noise_impl()
    _NOISE_CACHE.append(g)
    return g


def _gumbel_noise_impl():
    try:
        with tempfile.TemporaryDirectory() as td:
            path = os.path.join(td, "g.npy")
            env = dict(os.environ)
            env["PYTHONPATH"] = ""
            env["JAX_PLATFORMS"] = "cpu"
            r = subprocess.run(
                [sys.executable, "-c", _GUMBEL_SUBPROC, path],
                env=env, capture_output=True, timeout=300,
            )
            if r.returncode == 0 and os.path.exists(path):
                g = np.load(path)
                if g.shape == (S, B, S) and g.dtype == np.float32:
                    return g
    except Exception:
        pass
    return _gumbel_numpy()


# ----------------------------------------------------------------------------
# Bass program (SPMD, one build shared by all 8 cores)
# ----------------------------------------------------------------------------

_COMPILED = {}


def _patch_tile_drain():
    """walrus rejects the final Tile drain when it carries >1 sem wait
    ('Too many sync wait commands'); spread waits over single-wait SP NOPs."""
    import concourse.tile as tile_mod
    import concourse.mybir as mybir
    from concourse.tile import ScopedClock

    if getattr(tile_mod.TileContext, "_drain_patched", False):
        return

    def _patched(self, tick_clock, wait_clock):
        drain_inst = self.nc.sync.drain()
        wait_clock.add_sem_waits(
            drain_inst.ins, ScopedClock({None: tick_clock.global_clock})
        )
        si = drain_inst.ins.sync_info
        waits = list(si.on_wait or []) if si is not None else []
        if len(waits) > 1:
            si.on_wait = [waits[0]]
            for w in waits[1:]:
                nop = self.nc.sync.nop()
                nsi = nop.ins.sync_info
                if nsi is None:
                    nop.ins.sync_info = mybir.SyncInfo(on_wait=[w], on_update=[])
                else:
                    nsi.on_wait = [w]
        self.nc.all_engine_barrier()
        assert self.sems is not None
        popped = self.nc._tile_sem_poison_stack.pop()
        assert popped is self._sem_poison
        self.nc.clear_and_free_semaphores(list(self.sems.allocated().values()))
        self.nc.all_engine_barrier()

    tile_mod.TileContext._drain_and_barrier = _patched
    tile_mod.TileContext._drain_patched = True


def _split_multiwaits(bir_bytes, max_waits=1):
    """walrus (this build) rejects instructions carrying more than one sem
    wait ('Too many sync wait commands'). Move excess waits onto NoOp
    instructions injected immediately before the offender (same engine)."""
    import orjson

    d = orjson.loads(bir_bytes)
    ctr = [0]

    def fix_block(bb):
        if not isinstance(bb, dict):
            return
        instrs = bb.get("instructions")
        if isinstance(instrs, list):
            out = []
            for ins in instrs:
                si = ins.get("sync_info") if isinstance(ins, dict) else None
                waits = si.get("on_wait") if si else None
                if waits and len(waits) > max_waits:
                    keep = waits[:max_waits]
                    extra = waits[max_waits:]
                    for w in extra:
                        ctr[0] += 1
                        out.append({
                            "debug": ins.get("debug"),
                            "engine": ins["engine"],
                            "ins": [], "outs": [],
                            "name": f"I-waitfix-{ctr[0]}",
                            "opcode": "NoOp",
                            "sync_info": {"on_wait": [w], "on_update": []},
                        })
                    si["on_wait"] = keep
                out.append(ins)
            bb["instructions"] = out
        for v in bb.values():
            fix_any(v)

    def fix_any(o):
        if isinstance(o, dict):
            if "instructions" in o:
                fix_block(o)
            else:
                for v in o.values():
                    fix_any(v)
        elif isinstance(o, list):
            for v in o:
                fix_any(v)

    for f in d.get("functions", []):
        fix_any(f)
    return orjson.dumps(d)


def _build_nc(bvp_val, reps=1):
    import concourse.bass as bass
    import concourse.mybir as mybir
    import concourse.tile as tile

    _patch_tile_drain()
    f32 = mybir.dt.float32
    u32 = mybir.dt.uint32
    Alu = mybir.AluOpType
    Act = mybir.ActivationFunctionType

    def mmd(ap):  # matmul dtype view
        return ap.bitcast(mybir.dt.float32r) if _F32R else ap

    nc = bass.Bass("TRN2", target_bir_lowering=False)

    # per-core inputs
    ccT_d = nc.dram_tensor("ccT", [E, BL * S], f32, kind="ExternalInput")
    gm_d = nc.dram_tensor("gm", [BL, S * S], f32, kind="ExternalInput")   # [b,(t,s)]
    mask0_d = nc.dram_tensor("mask0", [BL, S], f32, kind="ExternalInput")  # *1e8
    q0T_d = nc.dram_tensor("q0T", [H, BL], f32, kind="ExternalInput")
    cstT_d = nc.dram_tensor("cstT", [H, BL], f32, kind="ExternalInput")
    # shared params
    W2_d = nc.dram_tensor("W2", [E, H], f32, kind="ExternalInput")
    Wb1_d = nc.dram_tensor("Wb1", [E, H], f32, kind="ExternalInput")  # Wv_bot@W1
    Wt1_d = nc.dram_tensor("Wt1", [E, H], f32, kind="ExternalInput")  # Wv_top@W1
    b2_d = nc.dram_tensor("b2c", [H, 1], f32, kind="ExternalInput")
    wv_d = nc.dram_tensor("wvc", [H, 1], f32, kind="ExternalInput")
    rowb_d = nc.dram_tensor("rowb", [BL, 1], u32, kind="ExternalInput")  # b*64
    ident_d = nc.dram_tensor("ident", [BL, BL], f32, kind="ExternalInput")
    # outputs
    th_out_d = nc.dram_tensor("th_out", [BL, S * S], f32, kind="ExternalOutput")
    act_out_d = nc.dram_tensor("act_out", [BL, S], u32, kind="ExternalOutput")
    # internal DRAM gather tables [BL*S, H]
    Gb_d = nc.dram_tensor("Gb", [BL * S, H], f32)
    Gt_d = nc.dram_tensor("Gt", [BL * S, H], f32)

    with tile.TileContext(nc) as tc:
        with tc.tile_pool(name="pers", bufs=1) as pers:
            # ---- load persistent data ----
            cc_sb = pers.tile([E, BL * S], f32)
            nc.sync.dma_start(out=cc_sb[:], in_=ccT_d[:])
            gm_sb = pers.tile([BL, S * S], f32)
            nc.sync.dma_start(out=gm_sb[:], in_=gm_d[:])
            mask_sb = pers.tile([BL, S], f32)
            nc.sync.dma_start(out=mask_sb[:], in_=mask0_d[:])
            qT_sb = pers.tile([H, BL], f32)
            nc.sync.dma_start(out=qT_sb[:], in_=q0T_d[:])
            cstT_sb = pers.tile([H, BL], f32)
            nc.sync.dma_start(out=cstT_sb[:], in_=cstT_d[:])
            W2_sb = pers.tile([E, H], f32)
            nc.sync.dma_start(out=W2_sb[:], in_=W2_d[:])
            Wb1_sb = pers.tile([E, H], f32)
            nc.sync.dma_start(out=Wb1_sb[:], in_=Wb1_d[:])
            Wt1_sb = pers.tile([E, H], f32)
            nc.sync.dma_start(out=Wt1_sb[:], in_=Wt1_d[:])
            b2_sb = pers.tile([H, 1], f32)
            nc.sync.dma_start(out=b2_sb[:], in_=b2_d[:])
            wv_sb = pers.tile([H, 1], f32)
            nc.sync.dma_start(out=wv_sb[:], in_=wv_d[:])
            rowb_sb = pers.tile([BL, 1], u32)
            nc.sync.dma_start(out=rowb_sb[:], in_=rowb_d[:])
            ident_sb = pers.tile([BL, BL], f32)
            nc.sync.dma_start(out=ident_sb[:], in_=ident_d[:])

            tgt_sb = pers.tile([H, BL * S], f32)   # transposed tgt (+b2)
            th_sb = pers.tile([BL, S * S], f32)    # tanh(u+bvp) per step
            act8_sb = pers.tile([BL, 8 * S], u32)
            u_stage = pers.tile([1, BL * S], f32)  # u staging (psum->sbuf)

            # ---- setup: tgt_T = W2^T cc_T + b2 ; G tables to DRAM ----
            with tc.tile_pool(name="sps", bufs=2, space="PSUM") as sps, \
                 tc.tile_pool(name="ssb", bufs=3) as ssb:
                for k in range(8):
                    pt = sps.tile([H, 512], f32)
                    nc.tensor.matmul(out=pt[:], lhsT=mmd(W2_sb[:]),
                                     rhs=mmd(cc_sb[:, 512 * k:512 * (k + 1)]),
                                     start=True, stop=True)
                    nc.vector.tensor_scalar_add(
                        out=tgt_sb[:, 512 * k:512 * (k + 1)], in0=pt[:],
                        scalar1=b2_sb[:, 0:1])
                for j in range(32):
                    sl = slice(128 * j, 128 * (j + 1))
                    pg = sps.tile([128, H], f32)
                    nc.tensor.matmul(out=pg[:], lhsT=mmd(cc_sb[:, sl]),
                                     rhs=mmd(Wb1_sb[:]), start=True, stop=True)
                    gst = ssb.tile([128, H], f32)
                    nc.vector.tensor_copy(out=gst[:], in_=pg[:])
                    nc.sync.dma_start(out=Gb_d[sl, :], in_=gst[:])
                for j in range(32):
                    sl = slice(128 * j, 128 * (j + 1))
                    pg2 = sps.tile([128, H], f32)
                    nc.tensor.matmul(out=pg2[:], lhsT=mmd(cc_sb[:, sl]),
                                     rhs=mmd(Wt1_sb[:]), start=True, stop=True)
                    gst2 = ssb.tile([128, H], f32)
                    nc.vector.tensor_copy(out=gst2[:], in_=pg2[:])
                    nc.sync.dma_start(out=Gt_d[sl, :], in_=gst2[:])

            # ---- decode loop ----
            with tc.tile_pool(name="lsb", bufs=3) as lsb, \
                 tc.tile_pool(name="small", bufs=6) as sm, \
                 tc.tile_pool(name="psu", bufs=(1 if _MERGE else 6), space="PSUM") as psu, \
                 tc.tile_pool(name="psq", bufs=3, space="PSUM") as psq:
                tgt_v = tgt_sb[:].rearrange("p (b s) -> p b s", s=S)
                for rep in range(reps):
                  if rep > 0:  # re-init recurrent state (timing reps only)
                    nc.sync.dma_start(out=mask_sb[:], in_=mask0_d[:])
                    nc.sync.dma_start(out=qT_sb[:], in_=q0T_d[:])
                    nc.sync.dma_start(out=cstT_sb[:], in_=cstT_d[:])
                  for t in range(S):
                    # A = tgt + q (broadcast over s), T = tanh(A)
                    A = lsb.tile([H, BL * S], f32, tag="A")
                    T = lsb.tile([H, BL * S], f32, tag="T")
                    Av = A[:].rearrange("p (b s) -> p b s", s=S)
                    Tv = T[:].rearrange("p (b s) -> p b s", s=S)
                    gw = int(os.environ.get("KERNEL_GPS_BW", "0"))  # b-rows on gpsimd
                    if gw:
                        bs = slice(BL - gw, BL)
                        nc.gpsimd.tensor_tensor(
                            out=Av[:, bs, :], in0=tgt_v[:, bs, :],
                            in1=qT_sb[:, bs].to_broadcast([H, gw, S]),
                            op=Alu.add)
                        nc.scalar.activation(
                            out=Tv[:, bs, :], in_=Av[:, bs, :], func=Act.Tanh)
                    bw = (BL - gw) // _NSPLIT
                    for hb in range(_NSPLIT):
                        bs = slice(bw * hb, bw * (hb + 1))
                        nc.vector.tensor_tensor(
                            out=Av[:, bs, :], in0=tgt_v[:, bs, :],
                            in1=qT_sb[:, bs].to_broadcast([H, bw, S]),
                            op=Alu.add)
                        nc.scalar.activation(
                            out=Tv[:, bs, :], in_=Av[:, bs, :], func=Act.Tanh)
                    if os.environ.get("KERNEL_COMPUTE_ONLY") == "1":
                        continue
                    # u = wv^T T -> psum chunks -> SBUF stage -> DMA to [b, s]
                    u_sb = sm.tile([BL, S], f32, tag="u")
                    if _MERGE:
                        for k in range(2):
                            pu = psu.tile([1, 2048], f32, tag="pu")
                            for h2 in range(4):
                                c = 4 * k + h2
                                nc.tensor.matmul(
                                    out=pu[0:1, 512 * h2:512 * (h2 + 1)],
                                    lhsT=mmd(wv_sb[:]),
                                    rhs=mmd(T[:, 512 * c:512 * (c + 1)]),
                                    start=True, stop=True)
                            dst = u_stage[0:1, 2048 * k:2048 * (k + 1)]
                            if k % 2 == 0:
                                nc.scalar.copy(out=dst, in_=pu[:])
                            else:
                                nc.vector.tensor_copy(out=dst, in_=pu[:])
                        nc.sync.dma_start(
                            out=u_sb[:],
                            in_=u_stage[0:1, :].rearrange("p (b s) -> p b s", s=S))
                    else:
                        for k in range(8):
                            pu = psu.tile([1, 512], f32, tag="pu")
                            nc.tensor.matmul(out=pu[:], lhsT=mmd(wv_sb[:]),
                                             rhs=mmd(T[:, 512 * k:512 * (k + 1)]),
                                             start=True, stop=True)
                            dst = u_stage[0:1, 512 * k:512 * (k + 1)]
                            if (k * _ACT_COPIES) // 8 != ((k + 1) * _ACT_COPIES) // 8:
                                nc.scalar.copy(out=dst, in_=pu[:])
                            else:
                                nc.vector.tensor_copy(out=dst, in_=pu[:])
                        for hb in range(2):
                            nc.sync.dma_start(
                                out=u_sb[32 * hb:32 * (hb + 1), :],
                                in_=u_stage[0:1, 2048 * hb:2048 * (hb + 1)]
                                .rearrange("p (b s) -> p b s", s=S))
                    # th = tanh(u + bvp) -> persistent output buffer
                    tsl = slice(S * t, S * (t + 1))
                    nc.scalar.activation(out=th_sb[:, tsl], in_=u_sb[:],
                                         func=Act.Tanh, bias=float(bvp_val))
                    # y = 10*th + (g_t - mask)
                    gmm = sm.tile([BL, S], f32, tag="gmm")
                    nc.vector.tensor_tensor(out=gmm[:], in0=gm_sb[:, tsl],
                                            in1=mask_sb[:], op=Alu.subtract)
                    y = sm.tile([BL, S], f32, tag="y")
                    nc.vector.scalar_tensor_tensor(
                        out=y[:], in0=th_sb[:, tsl], scalar=float(C_CLIP),
                        in1=gmm[:], op0=Alu.mult, op1=Alu.add)
                    # sample: argmax over s
                    mx8 = sm.tile([BL, 8], f32, tag="mx8")
                    ix8 = act8_sb[:, 8 * t:8 * (t + 1)]
                    nc.vector.max(out=mx8[:], in_=y[:])
                    nc.vector.max_index(out=ix8, in_max=mx8[:], in_values=y[:])
                    # onehot + mask update
                    oh = sm.tile([BL, S], f32, tag="oh")
                    nc.vector.tensor_scalar(
                        out=oh[:], in0=y[:], scalar1=mx8[:, 0:1], scalar2=None,
                        op0=Alu.is_equal)
                    nc.vector.scalar_tensor_tensor(
                        out=mask_sb[:], in0=oh[:], scalar=float(MASK_NEG),
                        in1=mask_sb[:], op0=Alu.mult, op1=Alu.add)
                    # next-q gather
                    if (t < S - 1 or t == 0) and os.environ.get("KERNEL_NOGATHER") != "1":
                        fl = sm.tile([BL, 1], u32, tag="fl")
                        nc.vector.tensor_tensor(out=fl[:], in0=ix8[:, 0:1],
                                                in1=rowb_sb[:], op=Alu.add)
                        if t == 0:
                            g0 = sm.tile([BL, H], f32, tag="g0")
                            nc.gpsimd.indirect_dma_start(
                                out=g0[:], out_offset=None, in_=Gt_d[:],
                                in_offset=bass.IndirectOffsetOnAxis(
                                    ap=fl[:, 0:1], axis=0))
                            pq0 = psq.tile([H, BL], f32, tag="pq")
                            nc.tensor.transpose(out=pq0[:], in_=g0[:],
                                                identity=ident_sb[:])
                            nc.vector.tensor_tensor(out=cstT_sb[:], in0=cstT_sb[:],
                                                    in1=pq0[:], op=Alu.add)
                        if t < S - 1:
                            gq = sm.tile([BL, H], f32, tag="gq")
                            nc.gpsimd.indirect_dma_start(
                                out=gq[:], out_offset=None, in_=Gb_d[:],
                                in_offset=bass.IndirectOffsetOnAxis(
                                    ap=fl[:, 0:1], axis=0))
                            pq = psq.tile([H, BL], f32, tag="pq")
                            nc.tensor.transpose(out=pq[:], in_=gq[:],
                                                identity=ident_sb[:])
                            nc.vector.tensor_tensor(out=qT_sb[:], in0=cstT_sb[:],
                                                    in1=pq[:], op=Alu.add)

            # ---- outputs ----
            if os.environ.get("KERNEL_COMPUTE_ONLY") == "1":
                nc.vector.memset(th_sb[:], 0.0)
                nc.vector.memset(act8_sb[:], 0)
            nc.sync.dma_start(out=th_out_d[:], in_=th_sb[:])
            nc.sync.dma_start(out=act_out_d[:],
                              in_=act8_sb[:].rearrange("p (t e) -> p t e", e=8)[:, :, 0])

    _orig_tjb = nc.to_json_bytes

    def _patched_tjb():
        return _split_multiwaits(_orig_tjb())

    nc.to_json_bytes = _patched_tjb
    return nc


# ----------------------------------------------------------------------------
# Host side
# ----------------------------------------------------------------------------

def _host_prep(inputs):
    f = np.float32
    cc = np.asarray(inputs["cell_context"], f)
    hm = np.asarray(inputs["high_mask"], f)
    Wc, bc = np.asarray(inputs["Wc"], f), np.asarray(inputs["bc"], f)
    Wv, bv = np.asarray(inputs["Wv"], f), np.asarray(inputs["bv"], f)
    W1, b1 = np.asarray(inputs["W1"], f), np.asarray(inputs["b1"], f)
    W2, b2 = np.asarray(inputs["W2"], f), np.asarray(inputs["b2"], f)
    wv = np.asarray(inputs["wv"], f)
    init_w = np.asarray(inputs["init_w"], f)
    bvp = float(np.asarray(inputs["bvp"], f))

    h_mean = cc.mean(axis=1, dtype=f).astype(f)
    h_bar = ((h_mean @ Wc).astype(f) + bc).astype(f)
    query0 = (h_bar + ((init_w @ Wv).astype(f) + bv)).astype(f)
    q0 = ((query0 @ W1).astype(f) + b1).astype(f)
    cst = (((h_bar + bv).astype(f) @ W1).astype(f) + b1).astype(f)
    Wb1 = (Wv[E:] @ W1).astype(f)
    Wt1 = (Wv[:E] @ W1).astype(f)

    g = _gumbel_noise()  # [S, B, S]

    shared = {
        "W2": np.ascontiguousarray(W2),
        "Wb1": np.ascontiguousarray(Wb1),
        "Wt1": np.ascontiguousarray(Wt1),
        "b2c": np.ascontiguousarray(b2.reshape(H, 1)),
        "wvc": np.ascontiguousarray(wv.reshape(H, 1)),
        "rowb": (np.arange(BL, dtype=np.uint32) * S).reshape(BL, 1),
        "ident": np.eye(BL, dtype=f),
    }
    in_maps = []
    for c in range(NC):
        bs = slice(BL * c, BL * (c + 1))
        cc_l = cc[bs]  # [BL, S, E]
        m = dict(shared)
        m["ccT"] = np.ascontiguousarray(cc_l.reshape(BL * S, E).T)
        m["gm"] = np.ascontiguousarray(
            g[:, bs, :].transpose(1, 0, 2).reshape(BL, S * S))
        m["mask0"] = np.ascontiguousarray(hm[bs] * MASK_NEG)
        m["q0T"] = np.ascontiguousarray(q0[bs].T)
        m["cstT"] = np.ascontiguousarray(cst[bs].T)
        in_maps.append(m)
    return in_maps, g, bvp


def _host_finish(inputs, th_all, act_all):
    """th_all [B, S, S] tanh(u+bvp); act_all [B, S] int. Compute logp/reward."""
    f = np.float32
    od = np.asarray(inputs["original_data"], f)
    hm = np.asarray(inputs["high_mask"], f)
    bidx = np.arange(B)
    mask = hm.copy()
    logp = np.zeros(B, f)
    rew = np.zeros(B, f)
    last = np.zeros((B, 2), f)
    for t in range(S):
        idx = act_all[:, t]
        logits = (C_CLIP * th_all[:, t, :] - MASK_NEG * mask).astype(f)
        m = logits.max(axis=-1, keepdims=True)
        sh = (logits - m).astype(f)
        lse = (np.log(np.exp(sh, dtype=f).sum(axis=-1, dtype=f), dtype=f)
               + m[:, 0]).astype(f)
        logp = (logp + logits[bidx, idx] - lse).astype(f)
        node = od[bidx, idx]
        if t > 0:
            rew = (rew + np.sqrt(((node - last) ** 2).sum(-1, dtype=f),
                                 dtype=f)).astype(f)
        last = node
        mask = mask.copy()
        mask[bidx, idx] = 1.0
    return logp, rew


def kernel(**inputs):
    return _kernel_impl(inputs, reps=int(os.environ.get("KERNEL_REPS", "1")))


def _kernel_impl(inputs, reps=1):
    from concourse.bass_utils import run_bass_kernel_spmd

    in_maps, _g, bvp = _host_prep(inputs)

    key = ("nc", bvp, _F32R, reps)
    if key not in _COMPILED:
        _COMPILED[key] = _build_nc(bvp, reps)
    nc = _COMPILED[key]

    res = run_bass_kernel_spmd(
        nc, in_maps, core_ids=list(range(NC)),
        trace=os.environ.get("KERNEL_TRACE", "0") == "1",
    )

    th_all = np.zeros((B, S, S), np.float32)
    act_all = np.zeros((B, S), np.int64)
    for c in range(NC):
        r = res.results[c]
        th_all[BL * c:BL * (c + 1)] = r["th_out"].reshape(BL, S, S)
        act_all[BL * c:BL * (c + 1)] = r["act_out"].astype(np.int64)

    logp, rew = _host_finish(inputs, th_all, act_all)
    kernel._last_result = res  # for test harness profiling
    return logp, rew, act_all.astype(np.int32)


# revision 22
# speedup vs baseline: 1.1107x; 1.1107x over previous
"""Trainium2 Bass kernel for nn_Decoder: pointer-network sequential decode with
Gumbel-max categorical sampling, data-parallel over batch across 8 NeuronCores.

Contract: kernel(**inputs) takes FULL unsharded inputs (as produced by
setup_inputs()) and returns (cell_log_prob [B], cell_reward [B], cell_action [B,S])
matching the reference bit-for-bit on sampled actions.

Strategy:
 - The jax.random.categorical noise is data-independent: precompute the exact
   Gumbel noise host-side (jax-CPU subprocess for bit-exactness; numpy threefry
   fallback) and feed it to the device.
 - Per core (64 batch rows): the sequential 64-step decode runs on-device:
   A = tgt + q (DVE), T = tanh(A) (ACT), u = wv.T @ T (PE), redistribute u via
   DMA, sample via max/max_index, gather next-q rows via indirect DMA from a
   precomputed G = cc @ (Wv_bot @ W1) table, q = cst + gathered^T (PE transpose).
 - log-prob and reward are pure outputs (not in the recurrence); the device
   streams out tanh(u) per step and the host finishes log_softmax/reward in
   fp32 numpy.
"""

import os
import subprocess
import sys
import tempfile

import numpy as np

B, S, E, H = 512, 64, 128, 128
NC = 8
BL = B // NC  # 64 rows per core
C_CLIP = np.float32(10.0)
MASK_NEG = np.float32(1e8)

_F32R = os.environ.get("KERNEL_F32R", "0") == "1"  # full-rate fp32 matmuls
_NSPLIT = int(os.environ.get("KERNEL_NSPLIT", "1"))   # add/tanh split count
_ACT_COPIES = int(os.environ.get("KERNEL_ACT_COPIES", "4"))  # of 8 u-chunk copies
_MERGE = os.environ.get("KERNEL_MERGE", "1") == "1"


# ----------------------------------------------------------------------------
# Gumbel noise replication (bit-exact with jax.random.categorical's noise)
# ----------------------------------------------------------------------------

_GUMBEL_SUBPROC = r"""
import numpy as np, sys
import jax, jax.numpy as jnp
key = jax.random.key(1)
keys = jax.random.split(key, %d)
g = np.stack([np.asarray(jax.random.gumbel(keys[t], (%d, %d), jnp.float32))
              for t in range(%d)])
np.save(sys.argv[1], g)
""" % (S, B, S, S)


def _rotl(x, r):
    return ((x << np.uint32(r)) | (x >> np.uint32(32 - r))).astype(np.uint32)


def _threefry2x32(k0, k1, x0, x1):
    ks0, ks1 = np.uint32(k0), np.uint32(k1)
    ks2 = np.uint32(np.uint32(0x1BD11BDA) ^ ks0 ^ ks1)
    x0 = (x0 + ks0).astype(np.uint32)
    x1 = (x1 + ks1).astype(np.uint32)
    sched = [(ks1, ks2, 1), (ks2, ks0, 2), (ks0, ks1, 3), (ks1, ks2, 4), (ks2, ks0, 5)]
    rotsets = [(13, 15, 26, 6), (17, 29, 16, 24)]
    for i, (ka, kb, c) in enumerate(sched):
        for r in rotsets[i % 2]:
            x0 = (x0 + x1).astype(np.uint32)
            x1 = (_rotl(x1, r) ^ x0).astype(np.uint32)
        x0 = (x0 + ka).astype(np.uint32)
        x1 = (x1 + kb + np.uint32(c)).astype(np.uint32)
    return x0, x1


def _threefry_split(kd, n):
    # jax.random.split: bits = threefry_random_bits(key, 32, (n*2,)) -> reshape [n,2]
    bits = _random_bits32(kd, n * 2)
    return bits.reshape(n, 2)


def _random_bits32(kd, n):
    x0 = np.zeros(n, np.uint32)
    x1 = np.arange(n, dtype=np.uint32)
    r0, r1 = _threefry2x32(kd[0], kd[1], x0, x1)
    return (r0 ^ r1).astype(np.uint32)


def _gumbel_numpy():
    # key(1) -> key_data is [0, 1] (threefry_seed packs the int into 2 words)
    base = np.array([0, 1], dtype=np.uint32)
    keys = _threefry_split(base, S)
    out = np.empty((S, B, S), np.float32)
    tiny = np.float32(np.finfo(np.float32).tiny)
    for t in range(S):
        bits = _random_bits32(keys[t], B * S)
        fb = ((bits >> np.uint32(9)) | np.uint32(0x3F800000)).astype(np.uint32)
        f = fb.view(np.float32) - np.float32(1.0)
        u = np.maximum(tiny, (f * (np.float32(1.0) - tiny) + tiny).astype(np.float32))
        out[t] = (-np.log(-np.log(u, dtype=np.float32), dtype=np.float32)).reshape(B, S)
    return out


_NOISE_CACHE = []


def _gumbel_noise():
    """[S, B, S] float32 — exactly jax.random.gumbel(keys[t], (B, S)) on XLA-CPU."""
    if _NOISE_CACHE:
        return _NOISE_CACHE[0]
    g = _gumbel_noise_impl()
    _NOISE_CACHE.append(g)
    return g


def _gumbel_noise_impl():
    try:
        with tempfile.TemporaryDirectory() as td:
            path = os.path.join(td, "g.npy")
            env = dict(os.environ)
            env["PYTHONPATH"] = ""
            env["JAX_PLATFORMS"] = "cpu"
            r = subprocess.run(
                [sys.executable, "-c", _GUMBEL_SUBPROC, path],
                env=env, capture_output=True, timeout=300,
            )
            if r.returncode == 0 and os.path.exists(path):
                g = np.load(path)
                if g.shape == (S, B, S) and g.dtype == np.float32:
                    return g
    except Exception:
        pass
    return _gumbel_numpy()


# ----------------------------------------------------------------------------
# Bass program (SPMD, one build shared by all 8 cores)
# ----------------------------------------------------------------------------

_COMPILED = {}


def _patch_tile_drain():
    """walrus rejects the final Tile drain when it carries >1 sem wait
    ('Too many sync wait commands'); spread waits over single-wait SP NOPs."""
    import concourse.tile as tile_mod
    import concourse.mybir as mybir
    from concourse.tile import ScopedClock

    if getattr(tile_mod.TileContext, "_drain_patched", False):
        return

    def _patched(self, tick_clock, wait_clock):
        drain_inst = self.nc.sync.drain()
        wait_clock.add_sem_waits(
            drain_inst.ins, ScopedClock({None: tick_clock.global_clock})
        )
        si = drain_inst.ins.sync_info
        waits = list(si.on_wait or []) if si is not None else []
        if len(waits) > 1:
            si.on_wait = [waits[0]]
            for w in waits[1:]:
                nop = self.nc.sync.nop()
                nsi = nop.ins.sync_info
                if nsi is None:
                    nop.ins.sync_info = mybir.SyncInfo(on_wait=[w], on_update=[])
                else:
                    nsi.on_wait = [w]
        self.nc.all_engine_barrier()
        assert self.sems is not None
        popped = self.nc._tile_sem_poison_stack.pop()
        assert popped is self._sem_poison
        self.nc.clear_and_free_semaphores(list(self.sems.allocated().values()))
        self.nc.all_engine_barrier()

    tile_mod.TileContext._drain_and_barrier = _patched
    tile_mod.TileContext._drain_patched = True


def _split_multiwaits(bir_bytes, max_waits=1):
    """walrus (this build) rejects instructions carrying more than one sem
    wait ('Too many sync wait commands'). Move excess waits onto NoOp
    instructions injected immediately before the offender (same engine)."""
    import orjson

    d = orjson.loads(bir_bytes)
    ctr = [0]

    def fix_block(bb):
        if not isinstance(bb, dict):
            return
        instrs = bb.get("instructions")
        if isinstance(instrs, list):
            out = []
            for ins in instrs:
                si = ins.get("sync_info") if isinstance(ins, dict) else None
                waits = si.get("on_wait") if si else None
                if waits and len(waits) > max_waits:
                    keep = waits[:max_waits]
                    extra = waits[max_waits:]
                    for w in extra:
                        ctr[0] += 1
                        out.append({
                            "debug": ins.get("debug"),
                            "engine": ins["engine"],
                            "ins": [], "outs": [],
                            "name": f"I-waitfix-{ctr[0]}",
                            "opcode": "NoOp",
                            "sync_info": {"on_wait": [w], "on_update": []},
                        })
                    si["on_wait"] = keep
                out.append(ins)
            bb["instructions"] = out
        for v in bb.values():
            fix_any(v)

    def fix_any(o):
        if isinstance(o, dict):
            if "instructions" in o:
                fix_block(o)
            else:
                for v in o.values():
                    fix_any(v)
        elif isinstance(o, list):
            for v in o:
                fix_any(v)

    for f in d.get("functions", []):
        fix_any(f)
    return orjson.dumps(d)


def _build_nc(bvp_val, reps=1):
    import concourse.bass as bass
    import concourse.mybir as mybir
    import concourse.tile as tile

    _patch_tile_drain()
    f32 = mybir.dt.float32
    u32 = mybir.dt.uint32
    Alu = mybir.AluOpType
    Act = mybir.ActivationFunctionType

    def mmd(ap):  # matmul dtype view
        return ap.bitcast(mybir.dt.float32r) if _F32R else ap

    nc = bass.Bass("TRN2", target_bir_lowering=False)

    # per-core inputs
    ccT_d = nc.dram_tensor("ccT", [E, BL * S], f32, kind="ExternalInput")
    gm_d = nc.dram_tensor("gm", [BL, S * S], f32, kind="ExternalInput")   # [b,(t,s)]
    mask0_d = nc.dram_tensor("mask0", [BL, S], f32, kind="ExternalInput")  # *1e8
    q0T_d = nc.dram_tensor("q0T", [H, BL], f32, kind="ExternalInput")
    cstT_d = nc.dram_tensor("cstT", [H, BL], f32, kind="ExternalInput")
    # shared params
    W2_d = nc.dram_tensor("W2", [E, H], f32, kind="ExternalInput")
    Wb1_d = nc.dram_tensor("Wb1", [E, H], f32, kind="ExternalInput")  # Wv_bot@W1
    Wt1_d = nc.dram_tensor("Wt1", [E, H], f32, kind="ExternalInput")  # Wv_top@W1
    b2_d = nc.dram_tensor("b2c", [H, 1], f32, kind="ExternalInput")
    wv_d = nc.dram_tensor("wvc", [H, 1], f32, kind="ExternalInput")
    rowb_d = nc.dram_tensor("rowb", [BL, 1], u32, kind="ExternalInput")  # b*64
    ident_d = nc.dram_tensor("ident", [BL, BL], f32, kind="ExternalInput")
    # outputs
    th_out_d = nc.dram_tensor("th_out", [BL, S * S], f32, kind="ExternalOutput")
    act_out_d = nc.dram_tensor("act_out", [BL, S], u32, kind="ExternalOutput")
    # internal DRAM gather tables [BL*S, H]
    Gb_d = nc.dram_tensor("Gb", [BL * S, H], f32)
    Gt_d = nc.dram_tensor("Gt", [BL * S, H], f32)

    with tile.TileContext(nc) as tc:
        with tc.tile_pool(name="pers", bufs=1) as pers:
            # ---- load persistent data ----
            cc_sb = pers.tile([E, BL * S], f32)
            nc.sync.dma_start(out=cc_sb[:], in_=ccT_d[:])
            gm_sb = pers.tile([BL, S * S], f32)
            nc.sync.dma_start(out=gm_sb[:], in_=gm_d[:])
            mask_sb = pers.tile([BL, S], f32)
            nc.sync.dma_start(out=mask_sb[:], in_=mask0_d[:])
            qT_sb = pers.tile([H, BL], f32)
            nc.sync.dma_start(out=qT_sb[:], in_=q0T_d[:])
            cstT_sb = pers.tile([H, BL], f32)
            nc.sync.dma_start(out=cstT_sb[:], in_=cstT_d[:])
            W2_sb = pers.tile([E, H], f32)
            nc.sync.dma_start(out=W2_sb[:], in_=W2_d[:])
            Wb1_sb = pers.tile([E, H], f32)
            nc.sync.dma_start(out=Wb1_sb[:], in_=Wb1_d[:])
            Wt1_sb = pers.tile([E, H], f32)
            nc.sync.dma_start(out=Wt1_sb[:], in_=Wt1_d[:])
            b2_sb = pers.tile([H, 1], f32)
            nc.sync.dma_start(out=b2_sb[:], in_=b2_d[:])
            wv_sb = pers.tile([H, 1], f32)
            nc.sync.dma_start(out=wv_sb[:], in_=wv_d[:])
            rowb_sb = pers.tile([BL, 1], u32)
            nc.sync.dma_start(out=rowb_sb[:], in_=rowb_d[:])
            ident_sb = pers.tile([BL, BL], f32)
            nc.sync.dma_start(out=ident_sb[:], in_=ident_d[:])

            tgt_sb = pers.tile([H, BL * S], f32)   # transposed tgt (+b2)
            th_sb = pers.tile([BL, S * S], f32)    # tanh(u+bvp) per step
            act8_sb = pers.tile([BL, 8 * S], u32)
            u_stage = pers.tile([1, BL * S], f32)  # u staging (psum->sbuf)

            # ---- setup: tgt_T = W2^T cc_T + b2 ; G tables to DRAM ----
            with tc.tile_pool(name="sps", bufs=2, space="PSUM") as sps, \
                 tc.tile_pool(name="ssb", bufs=3) as ssb:
                for k in range(8):
                    pt = sps.tile([H, 512], f32)
                    nc.tensor.matmul(out=pt[:], lhsT=mmd(W2_sb[:]),
                                     rhs=mmd(cc_sb[:, 512 * k:512 * (k + 1)]),
                                     start=True, stop=True)
                    nc.vector.tensor_scalar_add(
                        out=tgt_sb[:, 512 * k:512 * (k + 1)], in0=pt[:],
                        scalar1=b2_sb[:, 0:1])
                for j in range(32):
                    sl = slice(128 * j, 128 * (j + 1))
                    pg = sps.tile([128, H], f32)
                    nc.tensor.matmul(out=pg[:], lhsT=mmd(cc_sb[:, sl]),
                                     rhs=mmd(Wb1_sb[:]), start=True, stop=True)
                    gst = ssb.tile([128, H], f32)
                    nc.vector.tensor_copy(out=gst[:], in_=pg[:])
                    nc.sync.dma_start(out=Gb_d[sl, :], in_=gst[:])
                for j in range(32):
                    sl = slice(128 * j, 128 * (j + 1))
                    pg2 = sps.tile([128, H], f32)
                    nc.tensor.matmul(out=pg2[:], lhsT=mmd(cc_sb[:, sl]),
                                     rhs=mmd(Wt1_sb[:]), start=True, stop=True)
                    gst2 = ssb.tile([128, H], f32)
                    nc.vector.tensor_copy(out=gst2[:], in_=pg2[:])
                    nc.sync.dma_start(out=Gt_d[sl, :], in_=gst2[:])

            # ---- decode loop ----
            with tc.tile_pool(name="lsb", bufs=3) as lsb, \
                 tc.tile_pool(name="small", bufs=6) as sm, \
                 tc.tile_pool(name="psu", bufs=(1 if _MERGE else 6), space="PSUM") as psu, \
                 tc.tile_pool(name="psq", bufs=3, space="PSUM") as psq:
                tgt_v = tgt_sb[:].rearrange("p (b s) -> p b s", s=S)
                for rep in range(reps):
                  if rep > 0:  # re-init recurrent state (timing reps only)
                    nc.sync.dma_start(out=mask_sb[:], in_=mask0_d[:])
                    nc.sync.dma_start(out=qT_sb[:], in_=q0T_d[:])
                    nc.sync.dma_start(out=cstT_sb[:], in_=cstT_d[:])
                  for t in range(S):
                    # A = tgt + q (broadcast over s), T = tanh(A)
                    A = lsb.tile([H, BL * S], f32, tag="A")
                    T = lsb.tile([H, BL * S], f32, tag="T")
                    Av = A[:].rearrange("p (b s) -> p b s", s=S)
                    Tv = T[:].rearrange("p (b s) -> p b s", s=S)
                    gw = int(os.environ.get("KERNEL_GPS_BW", "0"))  # b-rows on gpsimd
                    if gw:
                        bs = slice(BL - gw, BL)
                        nc.gpsimd.tensor_tensor(
                            out=Av[:, bs, :], in0=tgt_v[:, bs, :],
                            in1=qT_sb[:, bs].to_broadcast([H, gw, S]),
                            op=Alu.add)
                        nc.scalar.activation(
                            out=Tv[:, bs, :], in_=Av[:, bs, :], func=Act.Tanh)
                    bw = (BL - gw) // _NSPLIT
                    for hb in range(_NSPLIT):
                        bs = slice(bw * hb, bw * (hb + 1))
                        nc.vector.tensor_tensor(
                            out=Av[:, bs, :], in0=tgt_v[:, bs, :],
                            in1=qT_sb[:, bs].to_broadcast([H, bw, S]),
                            op=Alu.add)
                        nc.scalar.activation(
                            out=Tv[:, bs, :], in_=Av[:, bs, :], func=Act.Tanh)
                    if os.environ.get("KERNEL_COMPUTE_ONLY") == "1":
                        continue
                    # u = wv^T T -> psum chunks -> tanh on stage -> DMA to [b, s]
                    tsl_pre = slice(S * t, S * (t + 1))
                    if _MERGE:
                        for k in range(2):
                            pu = psu.tile([1, 2048], f32, tag="pu")
                            for h2 in range(4):
                                c = 4 * k + h2
                                nc.tensor.matmul(
                                    out=pu[0:1, 512 * h2:512 * (h2 + 1)],
                                    lhsT=mmd(wv_sb[:]),
                                    rhs=mmd(T[:, 512 * c:512 * (c + 1)]),
                                    start=True, stop=True)
                            # tanh(u+bvp) directly on the staging row (same
                            # ACT stream as the copy -> no extra handoff)
                            nc.scalar.activation(
                                out=u_stage[0:1, 2048 * k:2048 * (k + 1)],
                                in_=pu[:], func=Act.Tanh, bias=float(bvp_val))
                        # redistribute delivers finished th into the output buf
                        nc.sync.dma_start(
                            out=th_sb[:, tsl_pre].rearrange("p s -> p s"),
                            in_=u_stage[0:1, :].rearrange("p (b s) -> p b s", s=S))
                    else:
                        u_sb = sm.tile([BL, S], f32, tag="u")
                        for k in range(8):
                            pu = psu.tile([1, 512], f32, tag="pu")
                            nc.tensor.matmul(out=pu[:], lhsT=mmd(wv_sb[:]),
                                             rhs=mmd(T[:, 512 * k:512 * (k + 1)]),
                                             start=True, stop=True)
                            dst = u_stage[0:1, 512 * k:512 * (k + 1)]
                            if (k * _ACT_COPIES) // 8 != ((k + 1) * _ACT_COPIES) // 8:
                                nc.scalar.copy(out=dst, in_=pu[:])
                            else:
                                nc.vector.tensor_copy(out=dst, in_=pu[:])
                        for hb in range(2):
                            nc.sync.dma_start(
                                out=u_sb[32 * hb:32 * (hb + 1), :],
                                in_=u_stage[0:1, 2048 * hb:2048 * (hb + 1)]
                                .rearrange("p (b s) -> p b s", s=S))
                        nc.scalar.activation(out=th_sb[:, tsl_pre], in_=u_sb[:],
                                             func=Act.Tanh, bias=float(bvp_val))
                    tsl = tsl_pre
                    # y = 10*th + (g_t - mask)
                    gmm = sm.tile([BL, S], f32, tag="gmm")
                    nc.vector.tensor_tensor(out=gmm[:], in0=gm_sb[:, tsl],
                                            in1=mask_sb[:], op=Alu.subtract)
                    y = sm.tile([BL, S], f32, tag="y")
                    nc.vector.scalar_tensor_tensor(
                        out=y[:], in0=th_sb[:, tsl], scalar=float(C_CLIP),
                        in1=gmm[:], op0=Alu.mult, op1=Alu.add)
                    # sample: argmax over s
                    mx8 = sm.tile([BL, 8], f32, tag="mx8")
                    ix8 = act8_sb[:, 8 * t:8 * (t + 1)]
                    nc.vector.max(out=mx8[:], in_=y[:])
                    nc.vector.max_index(out=ix8, in_max=mx8[:], in_values=y[:])
                    # onehot + mask update
                    oh = sm.tile([BL, S], f32, tag="oh")
                    nc.vector.tensor_scalar(
                        out=oh[:], in0=y[:], scalar1=mx8[:, 0:1], scalar2=None,
                        op0=Alu.is_equal)
                    nc.vector.scalar_tensor_tensor(
                        out=mask_sb[:], in0=oh[:], scalar=float(MASK_NEG),
                        in1=mask_sb[:], op0=Alu.mult, op1=Alu.add)
                    # next-q gather
                    if (t < S - 1 or t == 0) and os.environ.get("KERNEL_NOGATHER") != "1":
                        fl = sm.tile([BL, 1], u32, tag="fl")
                        nc.vector.tensor_tensor(out=fl[:], in0=ix8[:, 0:1],
                                                in1=rowb_sb[:], op=Alu.add)
                        if t == 0:
                            g0 = sm.tile([BL, H], f32, tag="g0")
                            nc.gpsimd.indirect_dma_start(
                                out=g0[:], out_offset=None, in_=Gt_d[:],
                                in_offset=bass.IndirectOffsetOnAxis(
                                    ap=fl[:, 0:1], axis=0))
                            pq0 = psq.tile([H, BL], f32, tag="pq")
                            nc.tensor.transpose(out=pq0[:], in_=g0[:],
                                                identity=ident_sb[:])
                            nc.vector.tensor_tensor(out=cstT_sb[:], in0=cstT_sb[:],
                                                    in1=pq0[:], op=Alu.add)
                        if t < S - 1:
                            gq = sm.tile([BL, H], f32, tag="gq")
                            nc.gpsimd.indirect_dma_start(
                                out=gq[:], out_offset=None, in_=Gb_d[:],
                                in_offset=bass.IndirectOffsetOnAxis(
                                    ap=fl[:, 0:1], axis=0))
                            pq = psq.tile([H, BL], f32, tag="pq")
                            nc.tensor.transpose(out=pq[:], in_=gq[:],
                                                identity=ident_sb[:])
                            nc.vector.tensor_tensor(out=qT_sb[:], in0=cstT_sb[:],
                                                    in1=pq[:], op=Alu.add)

            # ---- outputs ----
            if os.environ.get("KERNEL_COMPUTE_ONLY") == "1":
                nc.vector.memset(th_sb[:], 0.0)
                nc.vector.memset(act8_sb[:], 0)
            nc.sync.dma_start(out=th_out_d[:], in_=th_sb[:])
            nc.sync.dma_start(out=act_out_d[:],
                              in_=act8_sb[:].rearrange("p (t e) -> p t e", e=8)[:, :, 0])

    _orig_tjb = nc.to_json_bytes

    def _patched_tjb():
        return _split_multiwaits(_orig_tjb())

    nc.to_json_bytes = _patched_tjb
    return nc


# ----------------------------------------------------------------------------
# Host side
# ----------------------------------------------------------------------------

def _host_prep(inputs):
    f = np.float32
    cc = np.asarray(inputs["cell_context"], f)
    hm = np.asarray(inputs["high_mask"], f)
    Wc, bc = np.asarray(inputs["Wc"], f), np.asarray(inputs["bc"], f)
    Wv, bv = np.asarray(inputs["Wv"], f), np.asarray(inputs["bv"], f)
    W1, b1 = np.asarray(inputs["W1"], f), np.asarray(inputs["b1"], f)
    W2, b2 = np.asarray(inputs["W2"], f), np.asarray(inputs["b2"], f)
    wv = np.asarray(inputs["wv"], f)
    init_w = np.asarray(inputs["init_w"], f)
    bvp = float(np.asarray(inputs["bvp"], f))

    h_mean = cc.mean(axis=1, dtype=f).astype(f)
    h_bar = ((h_mean @ Wc).astype(f) + bc).astype(f)
    query0 = (h_bar + ((init_w @ Wv).astype(f) + bv)).astype(f)
    q0 = ((query0 @ W1).astype(f) + b1).astype(f)
    cst = (((h_bar + bv).astype(f) @ W1).astype(f) + b1).astype(f)
    Wb1 = (Wv[E:] @ W1).astype(f)
    Wt1 = (Wv[:E] @ W1).astype(f)

    g = _gumbel_noise()  # [S, B, S]

    shared = {
        "W2": np.ascontiguousarray(W2),
        "Wb1": np.ascontiguousarray(Wb1),
        "Wt1": np.ascontiguousarray(Wt1),
        "b2c": np.ascontiguousarray(b2.reshape(H, 1)),
        "wvc": np.ascontiguousarray(wv.reshape(H, 1)),
        "rowb": (np.arange(BL, dtype=np.uint32) * S).reshape(BL, 1),
        "ident": np.eye(BL, dtype=f),
    }
    in_maps = []
    for c in range(NC):
        bs = slice(BL * c, BL * (c + 1))
        cc_l = cc[bs]  # [BL, S, E]
        m = dict(shared)
        m["ccT"] = np.ascontiguousarray(cc_l.reshape(BL * S, E).T)
        m["gm"] = np.ascontiguousarray(
            g[:, bs, :].transpose(1, 0, 2).reshape(BL, S * S))
        m["mask0"] = np.ascontiguousarray(hm[bs] * MASK_NEG)
        m["q0T"] = np.ascontiguousarray(q0[bs].T)
        m["cstT"] = np.ascontiguousarray(cst[bs].T)
        in_maps.append(m)
    return in_maps, g, bvp


def _host_finish(inputs, th_all, act_all):
    """th_all [B, S, S] tanh(u+bvp); act_all [B, S] int. Compute logp/reward."""
    f = np.float32
    od = np.asarray(inputs["original_data"], f)
    hm = np.asarray(inputs["high_mask"], f)
    bidx = np.arange(B)
    mask = hm.copy()
    logp = np.zeros(B, f)
    rew = np.zeros(B, f)
    last = np.zeros((B, 2), f)
    for t in range(S):
        idx = act_all[:, t]
        logits = (C_CLIP * th_all[:, t, :] - MASK_NEG * mask).astype(f)
        m = logits.max(axis=-1, keepdims=True)
        sh = (logits - m).astype(f)
        lse = (np.log(np.exp(sh, dtype=f).sum(axis=-1, dtype=f), dtype=f)
               + m[:, 0]).astype(f)
        logp = (logp + logits[bidx, idx] - lse).astype(f)
        node = od[bidx, idx]
        if t > 0:
            rew = (rew + np.sqrt(((node - last) ** 2).sum(-1, dtype=f),
                                 dtype=f)).astype(f)
        last = node
        mask = mask.copy()
        mask[bidx, idx] = 1.0
    return logp, rew


def kernel(**inputs):
    return _kernel_impl(inputs, reps=int(os.environ.get("KERNEL_REPS", "1")))


def _kernel_impl(inputs, reps=1):
    from concourse.bass_utils import run_bass_kernel_spmd

    in_maps, _g, bvp = _host_prep(inputs)

    key = ("nc", bvp, _F32R, reps)
    if key not in _COMPILED:
        _COMPILED[key] = _build_nc(bvp, reps)
    nc = _COMPILED[key]

    res = run_bass_kernel_spmd(
        nc, in_maps, core_ids=list(range(NC)),
        trace=os.environ.get("KERNEL_TRACE", "0") == "1",
    )

    th_all = np.zeros((B, S, S), np.float32)
    act_all = np.zeros((B, S), np.int64)
    for c in range(NC):
        r = res.results[c]
        th_all[BL * c:BL * (c + 1)] = r["th_out"].reshape(BL, S, S)
        act_all[BL * c:BL * (c + 1)] = r["act_out"].astype(np.int64)

    logp, rew = _host_finish(inputs, th_all, act_all)
    kernel._last_result = res  # for test harness profiling
    return logp, rew, act_all.astype(np.int32)
